# revision 37
# baseline (speedup 1.0000x reference)
"""Trainium2 Bass kernel for nn_BasePointProcess (spatiotemporal Hawkes loglik).

Contract: kernel(**inputs) takes FULL inputs (X [32,256,3], scalars), shards
batch across 8 NeuronCores (4 per core), runs one SPMD Bass program, gathers
FULL outputs (lams [32,256], loglik [32]).

Math (per batch b):
  part1: lams_i = softplus(alpha * sum_{j<i, t_j>0} exp(-beta dt - gamma d2) + mu) + 1e-5
         via rank-3 matmul C = 2g*(s1_i s1_j + s2_i s2_j) + bj'  (bj' scaled by 1/2g),
         W = exp(2g*C + a_i), masked-sum over j.
  part2: integral = sum_{t,g1,g2} softplus(mu + sum_s TK[s,t]*E1[s,g1]*E2[s,g2])
         E1/E2 separable 1-D Gaussians; Hadamard Ksp = E1 (x) E2 via repeat-AP
         tensor_tensor (split DVE/GPSIMD); f32r matmuls with two batches
         column-tiled in the PE array; fused Softplus+accum on ACT.
  loglik_b = sum_i ln(lams_i)*(t_i>0) - integral_b / 64^3
"""

import sys

sys.path.insert(0, "/opt/trn_rl_repo")

from contextlib import ExitStack

import numpy as np

import concourse.bass as bass
import concourse.tile as tile
from concourse import mybir
from concourse.bass import AP
from concourse.bass_utils import run_bass_kernel_spmd

F32 = mybir.dt.float32
F32R = mybir.dt.float32r
BF16 = mybir.dt.bfloat16
ALU = mybir.AluOpType
ACT = mybir.ActivationFunctionType

R = 64                     # INT_RES
B, S = 32, 256
NCORES = 8
BL = B // NCORES           # 4 batches per core
G = R * R
UNIT_VOL = 1.0 / (R ** 3)

# cst column layout
C_SSG = 0            # [128,64] ss grid bcast
C_EG2 = 64           # [128,128] exp(-g*ss^2) twice
C_TTG = 192          # [128,64] time grid bcast
C_ATK = 256          # [128,64] alpha*exp(-beta*tc) bcast
C_CAUS = 320         # [128,512] causal masks chunk0|chunk1
C_SEL = 832          # [128,2] partition-half selectors
NCST = 834

# fraction of Hadamard quarter-ops routed to gpsimd (load balance)
POOL_SHARE = True


def _rep_inner(ap: AP, n: int) -> AP:
    """[P, F] -> [P, F, n] view: each element repeated n times (inner step 0)."""
    return AP(ap.tensor, ap.offset, ap.ap + [[0, n]])


def _rep_outer(ap: AP, n: int) -> AP:
    """[P, F] -> [P, n, F] view: whole row repeated n times (outer step 0)."""
    return AP(ap.tensor, ap.offset, [ap.ap[0], [0, n]] + ap.ap[1:])


def _split_free(ap: AP, outer: int, inner: int) -> AP:
    """[P, outer*inner] dense -> [P, outer, inner] view."""
    return AP(ap.tensor, ap.offset, [ap.ap[0], [inner, outer], [1, inner]])


def build_program(mu: float, al: float, be: float, ga: float,
                  split_waits: bool = True):
    nc = bass.Bass("TRN2", target_bir_lowering=False, debug=False)

    inall = nc.dram_tensor("inall", [128, NCST + 24], F32, kind="ExternalInput")
    xp3 = nc.dram_tensor("xp3", [3, 2176], F32R, kind="ExternalInput")
    lams8 = nc.dram_tensor("lams8", [128, 2 * BL], F32, kind="ExternalOutput")
    ll4 = nc.dram_tensor("ll4", [1, BL], F32, kind="ExternalOutput")

    with tile.TileContext(nc) as tc:
        with ExitStack() as ctx:
            _body(ctx, tc, inall, xp3, lams8, ll4, mu, al, be, ga)
    if split_waits:
        _split_waits(nc)
    return nc


# ISA sync-wait slots are limited (DVE/PE tensor ops: 1, ACT: 2). Tile can
# emit more; hoist the excess into standalone wait instructions just before.
_WAIT_LIMIT = {"InstEventSemaphore": 1,
               "InstUnconditionalBranch": 99, "InstCall": 99}


def _split_waits(nc):
    n = [0]
    for f in nc.m.functions:
        for blk in f.blocks:
            out = []
            for inst in blk.instructions:
                si = getattr(inst, "sync_info", None)
                waits = list(si.on_wait) if si is not None and si.on_wait else []
                lim = _WAIT_LIMIT.get(type(inst).__name__, 1)
                if len(waits) > lim:
                    keep = waits[-lim:]
                    for w in waits[:-lim]:
                        n[0] += 1
                        out.append(mybir.InstEventSemaphore(
                            name=f"WSPLIT-{n[0]}",
                            engine=inst.engine,
                            ins=[], outs=[],
                            sync_info=mybir.SyncInfo(on_wait=[w], on_update=[]),
                        ))
                    si.on_wait = keep
                out.append(inst)
            blk.instructions = out


def _body(ctx, tc, inall, xp3, lams8, ll4, mu, al, be, ga):
    nc = tc.nc
    pc = ctx.enter_context(tc.tile_pool(name="const", bufs=1))
    ps = ctx.enter_context(tc.tile_pool(name="small", bufs=1))
    pw = ctx.enter_context(tc.tile_pool(name="wtile", bufs=3))
    pe12 = ctx.enter_context(tc.tile_pool(name="e12", bufs=8))
    ptk1 = ctx.enter_context(tc.tile_pool(name="tk1", bufs=8))
    pksp = ctx.enter_context(tc.tile_pool(name="ksp", bufs=8))
    pspo = ctx.enter_context(tc.tile_pool(name="spout", bufs=2))
    pps1 = ctx.enter_context(tc.tile_pool(name="psum1", bufs=1, space="PSUM"))
    pps2 = ctx.enter_context(tc.tile_pool(name="psum2", bufs=2, space="PSUM"))
    ppsm = ctx.enter_context(tc.tile_pool(name="psmini", bufs=2, space="PSUM"))
    ppsl = ctx.enter_context(tc.tile_pool(name="psuml", bufs=1, space="PSUM"))

    # ---- constants + data: exactly 2 input DMAs ---------------------------
    CST = pc.tile([128, NCST + 24], F32)
    nc.sync.dma_start(CST[:, :], inall[:, :])
    ssg = CST[:, C_SSG:C_SSG + R]
    eg2 = CST[:, C_EG2:C_EG2 + 2 * R]
    ttg = CST[:, C_TTG:C_TTG + R]
    atk = CST[:, C_ATK:C_ATK + R]
    TP = CST[:, NCST:NCST + 8]
    S1P = CST[:, NCST + 8:NCST + 16]
    S2P = CST[:, NCST + 16:NCST + 24]

    # XP3 [3, 2048]: cols 0:1024 = L3 (s1g;s2g;ones), 1024: = rh per b
    XP3 = ps.tile([3, 2176], F32R)
    nc.sync.dma_start(XP3[:, :], xp3[:, :])

    SEL = ps.tile([128, 2], F32)
    nc.vector.tensor_copy(SEL[:, :], CST[:, C_SEL:C_SEL + 2])
    sel2 = SEL[:, :]
    ONES = ps.tile([128, 1], F32)
    nc.vector.memset(ONES[:, :], 1.0)
    NMUB = ps.tile([128, 1], F32)
    nc.vector.memset(NMUB[:, :], -mu)

    # ---- small per-(b,c) scalars on DVE [128, 8] --------------------------
    P1Q = ps.tile([128, 8], F32)
    P2Q = ps.tile([128, 8], F32)
    SSQ = ps.tile([128, 8], F32)
    E1B = ps.tile([128, 8], F32)
    E2B = ps.tile([128, 8], F32)
    E1S = ps.tile([128, 8], F32)
    E2S = ps.tile([128, 8], F32)
    AI = ps.tile([128, 8], F32)
    VP = ps.tile([128, 8], F32)
    EBT = ps.tile([128, 8], F32)
    EV = ps.tile([128, 8], F32)
    nc.vector.tensor_tensor(P1Q[:, :], S1P, S1P, ALU.mult)
    nc.vector.tensor_tensor(P2Q[:, :], S2P, S2P, ALU.mult)
    nc.vector.tensor_tensor(SSQ[:, :], P1Q[:, :], P2Q[:, :], ALU.add)
    nc.vector.tensor_scalar_mul(E1B[:, :], P1Q[:, :], -ga)
    nc.vector.tensor_scalar_mul(E2B[:, :], P2Q[:, :], -ga)
    nc.vector.tensor_scalar_mul(E1S[:, :], S1P, 2.0 * ga)
    nc.vector.tensor_scalar_mul(E2S[:, :], S2P, 2.0 * ga)
    # AI = -be*TP - ga*SSQ
    nc.vector.tensor_scalar_mul(AI[:, :], SSQ[:, :], -ga)
    nc.vector.scalar_tensor_tensor(AI[:, :], TP, -be, AI[:, :], ALU.mult, ALU.add)
    nc.vector.tensor_scalar(VP[:, :], TP, 0.0, None, ALU.is_gt)
    nc.scalar.activation(EBT[:, :], TP, ACT.Exp, scale=be)
    nc.vector.tensor_tensor(EV[:, :], EBT[:, :], VP[:, :], ALU.mult)
    FJ = ps.tile([128, 1], F32)
    nc.scalar.activation(FJ[:, :], EV[:, 7:8], ACT.Identity)

    RH = [XP3[:, 1024 + b * S:1024 + (b + 1) * S] for b in range(BL)]

    KSUMA = ps.tile([128, 8], F32)
    KSUMB = ps.tile([128, BL], F32)

    # ---- part-1: W = exp(2g*C + a_i), block-causal masked sums -------------
    # c=0: only j<128 can be causal (triangular block); c=1: j<128 all-causal
    # (exp-accum, no mask), j in [128,256) triangular.
    for b in range(BL):
        for c in range(2):
            col = b * 2 + c
            pp1 = pps1.tile([128, S], F32)
            nc.tensor.matmul(
                pp1[:, :],
                XP3[:, col * 128:(col + 1) * 128],
                RH[b],
                start=True, stop=True)
            if c == 0:
                wt = pw.tile([128, 128], F32)
                nc.scalar.activation(
                    wt[:, :], pp1[:, 0:128], ACT.Exp,
                    scale=2.0 * ga, bias=AI[:, col:col + 1])
                wj = pw.tile([128, 128], F32, tag="wjunk")
                nc.vector.scalar_tensor_tensor(
                    wj[:, :], wt[:, :], 1.0,
                    CST[:, C_CAUS + 0:C_CAUS + 128],
                    ALU.mult, ALU.mult,
                    accum_out=KSUMA[:, col:col + 1])
            else:
                wl = pw.tile([128, 128], F32, tag="wleft")
                nc.scalar.activation(
                    wl[:, :], pp1[:, 0:128], ACT.Exp,
                    scale=2.0 * ga, bias=AI[:, col:col + 1],
                    accum_out=KSUMA[:, col:col + 1])
                wt = pw.tile([128, 128], F32)
                nc.scalar.activation(
                    wt[:, :], pp1[:, 128:256], ACT.Exp,
                    scale=2.0 * ga, bias=AI[:, col:col + 1])
                wj = pw.tile([128, 128], F32, tag="wjunk")
                nc.vector.scalar_tensor_tensor(
                    wj[:, :], wt[:, :], 1.0,
                    CST[:, C_CAUS + S + 128:C_CAUS + 2 * S],
                    ALU.mult, ALU.mult,
                    accum_out=KSUMB[:, b:b + 1])

    # ---- part-2 E12 / TK builds (with free row-sum accumulators) ----------
    E1SM = ps.tile([128, 8], F32)
    E2SM = ps.tile([128, 8], F32)
    TKS = ps.tile([128, 8], F32)
    E12 = {}
    TKT = {}
    for b in range(BL):
        for c in range(2):
            col = b * 2 + c
            pm = ppsm.tile([128, 2 * R], F32)
            nc.tensor.matmul(
                pm[:, :], XP3[:, col * 128:(col + 1) * 128],
                XP3[:, 2048:2176], start=True, stop=True)
            e = pe12.tile([128, 2 * R], F32)
            nc.scalar.activation(
                e[:, 0:R], pm[:, 0:R], ACT.Exp,
                bias=E1B[:, col:col + 1], accum_out=E1SM[:, col:col + 1])
            nc.scalar.activation(
                e[:, R:2 * R], pm[:, R:2 * R], ACT.Exp,
                bias=E2B[:, col:col + 1], accum_out=E2SM[:, col:col + 1])
            E12[(b, c)] = e

            tk1 = ptk1.tile([128, R], F32)
            nc.vector.tensor_scalar(
                tk1[:, :], ttg, TP[:, col:col + 1], EV[:, col:col + 1],
                ALU.is_ge, ALU.mult)
            tkt = ps.tile([128, R], BF16, tag=f"tkt{col}")
            nc.vector.scalar_tensor_tensor(
                tkt[:, :], tk1[:, :], 1.0, atk, ALU.mult, ALU.mult,
                accum_out=TKS[:, col:col + 1])
            TKT[(b, c)] = tkt

    # ---- Hadamard Ksp[s, (g1,g2)] = E1[s,g1]*E2[s,g2], quarter-split ------
    POOL_Q = {(k * 32) // 13 for k in range(13)} if POOL_SHARE else set()
    KSP = {}
    for b in range(BL):
        for c in range(2):
            e = E12[(b, c)]
            kt = pksp.tile([128, G], BF16)
            for q in range(4):
                sl = kt[:, q * 1024:(q + 1) * 1024]
                out3 = AP(sl.tensor, sl.offset, [sl.ap[0], [R, 16], [1, R]])
                in_e1 = _rep_inner(e[:, q * 16:(q + 1) * 16], R)
                in_e2 = _rep_outer(e[:, R:2 * R], 16)
                i = (b * 2 + c) * 4 + q
                eng = nc.gpsimd if (i % 32) in POOL_Q else nc.vector
                eng.tensor_tensor(out3, in_e1, in_e2, ALU.mult)
            KSP[(b, c)] = kt

    # ---- part-2 matmuls + fused softplus-accum ----------------------------
    SPS = ps.tile([128, 2], F32)
    for pair in range(2):
        eo = pspo.tile([128, G], F32, tag="eo")
        for q in range(4):
            pp = pps2.tile([128, 1024], F32)
            # pair (i=0, i=1) always on different banks AND different PE
            # column groups -> concurrent; psum groups sequential per bank
            for phase in range(2):
                for c in range(2):
                    for i in range(2):
                        b = pair * 2 + i
                        ns = i ^ phase
                        off = q * 1024 + ns * 512
                        nc.tensor.matmul(
                            pp[i * 64:(i + 1) * 64, ns * 512:(ns + 1) * 512],
                            TKT[(b, c)][:, :],
                            KSP[(b, c)][:, off:off + 512],
                            start=(c == 0), stop=(c == 1),
                            tile_position=(0, i * 64))
            # softplus(z+mu) = (z+mu) + ln(1+exp(-(z+mu))); linear part
            # summed analytically via TKS/E1SM/E2SM below
            nc.scalar.activation(eo[:, q * 1024:(q + 1) * 1024], pp[:, :],
                                 ACT.Exp, scale=-1.0, bias=NMUB[:, :])
        lo = pspo.tile([128, G], F32, tag="lnout")
        nc.scalar.activation(
            lo[:, :], eo[:, :], ACT.Ln, bias=1.0,
            accum_out=SPS[:, pair:pair + 1])

    # ---- part-1 finish: lams = x + ln(1+exp(-x)) + 1e-5, x = al*Ksum+mu ---
    X1 = ps.tile([128, 8], F32)
    nc.vector.tensor_scalar(X1[:, :], KSUMA[:, :], al, mu, ALU.mult, ALU.add)
    x1o = X1[:, 1:2]
    x1odd = AP(x1o.tensor, x1o.offset, [x1o.ap[0], [2, BL]])
    nc.vector.scalar_tensor_tensor(
        x1odd, KSUMB[:, :], al, x1odd, ALU.mult, ALU.add)
    EN1 = ps.tile([128, 8], F32)
    nc.scalar.activation(EN1[:, :], X1[:, :], ACT.Exp, scale=-1.0)
    LN1 = ps.tile([128, 8], F32)
    nc.scalar.activation(LN1[:, :], EN1[:, :], ACT.Ln, bias=1.0)
    LAM = ps.tile([128, 8], F32)
    nc.vector.scalar_tensor_tensor(
        LAM[:, :], X1[:, :], 1e-5, LN1[:, :], ALU.add, ALU.add)
    nc.sync.dma_start(lams8[:, :], LAM[:, :])
    LNL = ps.tile([128, 8], F32)
    nc.scalar.activation(LNL[:, :], LAM[:, :], ACT.Ln)
    nc.vector.tensor_tensor(LNL[:, :], LNL[:, :], VP[:, :], ALU.mult)

    # zsum per (b,c): sum_s TKsum * E1sum * E2sum
    ZS = ps.tile([128, 8], F32)
    nc.vector.tensor_tensor(ZS[:, :], E1SM[:, :], E2SM[:, :], ALU.mult)
    nc.vector.tensor_tensor(ZS[:, :], ZS[:, :], TKS[:, :], ALU.mult)

    SLGS = ppsl.tile([1, 20], F32)
    nc.tensor.matmul(SLGS[0:1, 0:8], ONES[:, :], LNL[:, :], start=True, stop=True)
    nc.tensor.matmul(SLGS[0:1, 12:20], ONES[:, :], ZS[:, :], start=True, stop=True)

    # ---- grid sums: partition-half sums -----------------------------------
    for pair in range(2):
        nc.tensor.matmul(
            SLGS[0:1, 8 + 2 * pair:10 + 2 * pair],
            SPS[:, pair:pair + 1], sel2, start=True, stop=True)

    # ---- loglik = pairsum(SL) - GS*unit_vol -------------------------------
    SLSB = ps.tile([1, 20], F32)
    nc.vector.tensor_copy(SLSB[:, :], SLGS[0:1, :])

    def _pairadd(out_ap, base_off):
        a = SLSB[0:1, base_off:base_off + 8]
        ev = AP(a.tensor, a.offset, [a.ap[0], [2, 4]])
        od = AP(a.tensor, a.offset + 1, [a.ap[0], [2, 4]])
        nc.vector.tensor_tensor(out_ap, ev, od, ALU.add)

    SL4 = ps.tile([1, BL], F32)
    _pairadd(SL4[:, :], 0)          # sum_i ln(lam_i)*mask  per b
    ZS4 = ps.tile([1, BL], F32)
    _pairadd(ZS4[:, :], 12)         # sum z  per b
    # integral_b = ZS4 + mu*T*G + LNACC(b) ; ll = SL4 - UNIT_VOL*integral
    T1 = ps.tile([1, BL], F32)
    nc.vector.tensor_tensor(T1[:, :], ZS4[:, :], SLSB[0:1, 8:12], ALU.add)
    LL = ps.tile([1, BL], F32)
    nc.vector.scalar_tensor_tensor(
        LL[:, :], T1[:, :], -UNIT_VOL, SL4[:, :], ALU.mult, ALU.add)
    nc.vector.tensor_scalar_add(
        LL[:, :], LL[:, :], -UNIT_VOL * mu * float(R) * float(G))
    nc.sync.dma_start(ll4[:, :], LL[:, :])


# ---------------------------------------------------------------------------
_CACHE = {}


def make_inmaps(X: np.ndarray, mu, al, be, ga):
    ss = np.linspace(0.0, 1.0, R, dtype=np.float32)
    tg = np.linspace(0.0, 1.0, R, dtype=np.float32)
    cst = np.zeros((128, NCST), np.float32)
    cst[:, C_SSG:C_SSG + R] = ss
    eg2 = np.exp(-ga * ss * ss).astype(np.float32)
    cst[:, C_EG2:C_EG2 + 2 * R] = np.concatenate([eg2, eg2])
    cst[:, C_TTG:C_TTG + R] = tg
    cst[:, C_ATK:C_ATK + R] = (al * np.exp(-be * tg)).astype(np.float32)
    p = np.arange(128)
    for c in range(2):
        i_idx = c * 128 + p
        cst[:, C_CAUS + c * S:C_CAUS + (c + 1) * S] = (
            np.arange(S)[None, :] < i_idx[:, None]).astype(np.float32)
    cst[:, C_SEL] = (p < 64).astype(np.float32)
    cst[:, C_SEL + 1] = (p >= 64).astype(np.float32)

    in_maps = []
    for k in range(NCORES):
        xk = X[k * BL:(k + 1) * BL]                  # [4, 256, 3]
        xdat = np.ascontiguousarray(xk.transpose(0, 2, 1))  # [4, 3, 256]
        t, s1, s2 = xdat[:, 0], xdat[:, 1], xdat[:, 2]
        # inall = [cst | prt]; prt [128, 24] partition layouts, col = b*2+c
        inall = np.zeros((128, NCST + 24), np.float32)
        inall[:, :NCST] = cst
        for r, arr in ((0, t), (1, s1), (2, s2)):
            inall[:, NCST + r * 8:NCST + (r + 1) * 8] = (
                arr.reshape(BL, 2, 128).transpose(2, 0, 1).reshape(128, 8))
        # xp3 [3, 2048]: cols 0:1024 = [s1g; s2g; ones], 1024: = [s1;s2;bj'] per b
        bj = (be / (2.0 * ga)) * t - 0.5 * (s1 * s1 + s2 * s2) \
            - (1e9 / (2.0 * ga)) * (t <= 0)
        xp3 = np.ones((3, 2176), np.float32)
        xp3[0, :1024] = s1.reshape(-1)
        xp3[1, :1024] = s2.reshape(-1)
        xp3[0, 1024:2048] = s1.reshape(-1)
        xp3[1, 1024:2048] = s2.reshape(-1)
        xp3[2, 1024:2048] = bj.reshape(-1)
        xp3[0, 2048:2112] = 2.0 * ga * ss
        xp3[0, 2112:2176] = 0.0
        xp3[1, 2048:2112] = 0.0
        xp3[1, 2112:2176] = 2.0 * ga * ss
        xp3[2, 2048:2176] = np.tile(-ga * ss * ss, 2)
        in_maps.append({"inall": inall, "xp3": xp3})
    return in_maps


def kernel(X, mu, alpha, beta, gamma):
    X = np.asarray(X, dtype=np.float32)
    mu = float(mu); al = float(alpha); be = float(beta); ga = float(gamma)
    key = (mu, al, be, ga)
    if key not in _CACHE:
        _CACHE[key] = build_program(mu, al, be, ga)
    nc = _CACHE[key]
    in_maps = make_inmaps(X, mu, al, be, ga)
    res = run_bass_kernel_spmd(nc, in_maps, list(range(NCORES))).results
    lams = np.zeros((B, S), np.float32)
    ll = np.zeros((B,), np.float32)
    for k in range(NCORES):
        l8 = res[k]["lams8"]                         # [128, 8]
        lams[k * BL:(k + 1) * BL] = (
            l8.reshape(128, BL, 2).transpose(1, 2, 0).reshape(BL, S))
        ll[k * BL:(k + 1) * BL] = res[k]["ll4"][0]
    return lams, ll


# revision 38
# speedup vs baseline: 3.2825x; 3.2825x over previous
"""Trainium2 Bass kernel for nn_BasePointProcess (spatiotemporal Hawkes loglik).

Contract: kernel(**inputs) takes FULL inputs (X [32,256,3], scalars), shards
batch across 8 NeuronCores (4 per core), runs one SPMD Bass program, gathers
FULL outputs (lams [32,256], loglik [32]).

Math (per batch b):
  part1: lams_i = softplus(alpha * sum_{j<i, t_j>0} exp(-beta dt - gamma d2) + mu) + 1e-5
         via rank-3 matmul C = 2g*(s1_i s1_j + s2_i s2_j) + bj'  (bj' scaled by 1/2g),
         W = exp(2g*C + a_i), masked-sum over j.
  part2: integral = sum_{t,g1,g2} softplus(mu + sum_s TK[s,t]*E1[s,g1]*E2[s,g2])
         E1/E2 separable 1-D Gaussians; Hadamard Ksp = E1 (x) E2 via repeat-AP
         tensor_tensor (split DVE/GPSIMD); f32r matmuls with two batches
         column-tiled in the PE array; fused Softplus+accum on ACT.
  loglik_b = sum_i ln(lams_i)*(t_i>0) - integral_b / 64^3
"""

import sys

sys.path.insert(0, "/opt/trn_rl_repo")

from contextlib import ExitStack

import numpy as np

import concourse.bass as bass
import concourse.tile as tile
from concourse import mybir
from concourse.bass import AP
from concourse.bass_utils import run_bass_kernel_spmd

F32 = mybir.dt.float32
F32R = mybir.dt.float32r
BF16 = mybir.dt.bfloat16
ALU = mybir.AluOpType
ACT = mybir.ActivationFunctionType

R = 64                     # INT_RES
B, S = 32, 256
NCORES = 8
BL = B // NCORES           # 4 batches per core
G = R * R
UNIT_VOL = 1.0 / (R ** 3)

# cst column layout
C_SSG = 0            # [128,64] ss grid bcast
C_EG2 = 64           # [128,128] exp(-g*ss^2) twice
C_TTG = 192          # [128,64] time grid bcast
C_ATK = 256          # [128,64] alpha*exp(-beta*tc) bcast
C_CAUS = 320         # [128,512] causal masks chunk0|chunk1
C_SEL = 832          # [128,2] partition-half selectors
NCST = 834

# fraction of Hadamard quarter-ops routed to gpsimd (load balance)
POOL_SHARE = True


def _rep_inner(ap: AP, n: int) -> AP:
    """[P, F] -> [P, F, n] view: each element repeated n times (inner step 0)."""
    return AP(ap.tensor, ap.offset, ap.ap + [[0, n]])


def _rep_outer(ap: AP, n: int) -> AP:
    """[P, F] -> [P, n, F] view: whole row repeated n times (outer step 0)."""
    return AP(ap.tensor, ap.offset, [ap.ap[0], [0, n]] + ap.ap[1:])


def _split_free(ap: AP, outer: int, inner: int) -> AP:
    """[P, outer*inner] dense -> [P, outer, inner] view."""
    return AP(ap.tensor, ap.offset, [ap.ap[0], [inner, outer], [1, inner]])


def build_program(mu: float, al: float, be: float, ga: float,
                  split_waits: bool = True, loop: int = 0):
    nc = bass.Bass("TRN2", target_bir_lowering=False, debug=False)

    inall = nc.dram_tensor("inall", [128, NCST + 24], F32, kind="ExternalInput")
    xp3 = nc.dram_tensor("xp3", [3, 2176], F32R, kind="ExternalInput")
    lams8 = nc.dram_tensor("lams8", [128, 2 * BL], F32, kind="ExternalOutput")
    ll4 = nc.dram_tensor("ll4", [1, BL], F32, kind="ExternalOutput")

    with tile.TileContext(nc) as tc:
        with ExitStack() as ctx:
            if loop:
                with tc.For_i(0, loop, 1):
                    _body(ctx, tc, inall, xp3, lams8, ll4, mu, al, be, ga)
            else:
                _body(ctx, tc, inall, xp3, lams8, ll4, mu, al, be, ga)
    if split_waits:
        _split_waits(nc)
    return nc


# ISA sync-wait slots are limited (DVE/PE tensor ops: 1, ACT: 2). Tile can
# emit more; hoist the excess into standalone wait instructions just before.
_WAIT_LIMIT = {"InstEventSemaphore": 1,
               "InstUnconditionalBranch": 99, "InstCall": 99}


def _split_waits(nc):
    n = [0]
    for f in nc.m.functions:
        for blk in f.blocks:
            out = []
            for inst in blk.instructions:
                si = getattr(inst, "sync_info", None)
                waits = list(si.on_wait) if si is not None and si.on_wait else []
                lim = _WAIT_LIMIT.get(type(inst).__name__, 1)
                if len(waits) > lim:
                    keep = waits[-lim:]
                    for w in waits[:-lim]:
                        n[0] += 1
                        out.append(mybir.InstEventSemaphore(
                            name=f"WSPLIT-{n[0]}",
                            engine=inst.engine,
                            ins=[], outs=[],
                            sync_info=mybir.SyncInfo(on_wait=[w], on_update=[]),
                        ))
                    si.on_wait = keep
                out.append(inst)
            blk.instructions = out


def _body(ctx, tc, inall, xp3, lams8, ll4, mu, al, be, ga):
    nc = tc.nc
    pc = ctx.enter_context(tc.tile_pool(name="const", bufs=1))
    ps = ctx.enter_context(tc.tile_pool(name="small", bufs=1))
    pw = ctx.enter_context(tc.tile_pool(name="wtile", bufs=3))
    pe12 = ctx.enter_context(tc.tile_pool(name="e12", bufs=8))
    ptk1 = ctx.enter_context(tc.tile_pool(name="tk1", bufs=8))
    pksp = ctx.enter_context(tc.tile_pool(name="ksp", bufs=8))
    pspo = ctx.enter_context(tc.tile_pool(name="spout", bufs=2))
    pps1 = ctx.enter_context(tc.tile_pool(name="psum1", bufs=1, space="PSUM"))
    pps2 = ctx.enter_context(tc.tile_pool(name="psum2", bufs=2, space="PSUM"))
    ppsm = ctx.enter_context(tc.tile_pool(name="psmini", bufs=2, space="PSUM"))
    ppsl = ctx.enter_context(tc.tile_pool(name="psuml", bufs=1, space="PSUM"))

    # ---- constants + data: exactly 2 input DMAs ---------------------------
    CST = pc.tile([128, NCST + 24], F32)
    nc.sync.dma_start(CST[:, :], inall[:, :])
    ssg = CST[:, C_SSG:C_SSG + R]
    eg2 = CST[:, C_EG2:C_EG2 + 2 * R]
    ttg = CST[:, C_TTG:C_TTG + R]
    atk = CST[:, C_ATK:C_ATK + R]
    TP = CST[:, NCST:NCST + 8]
    S1P = CST[:, NCST + 8:NCST + 16]
    S2P = CST[:, NCST + 16:NCST + 24]

    # XP3 [3, 2048]: cols 0:1024 = L3 (s1g;s2g;ones), 1024: = rh per b
    XP3 = ps.tile([3, 2176], F32R)
    nc.sync.dma_start(XP3[:, :], xp3[:, :])

    SEL = ps.tile([128, 2], F32)
    nc.vector.tensor_copy(SEL[:, :], CST[:, C_SEL:C_SEL + 2])
    sel2 = SEL[:, :]
    ONES = ps.tile([128, 1], F32)
    nc.vector.memset(ONES[:, :], 1.0)
    NMUB = ps.tile([128, 1], F32)
    nc.vector.memset(NMUB[:, :], -mu)

    # ---- small per-(b,c) scalars on DVE [128, 8] --------------------------
    P1Q = ps.tile([128, 8], F32)
    P2Q = ps.tile([128, 8], F32)
    SSQ = ps.tile([128, 8], F32)
    E1B = ps.tile([128, 8], F32)
    E2B = ps.tile([128, 8], F32)
    E1S = ps.tile([128, 8], F32)
    E2S = ps.tile([128, 8], F32)
    AI = ps.tile([128, 8], F32)
    VP = ps.tile([128, 8], F32)
    EBT = ps.tile([128, 8], F32)
    EV = ps.tile([128, 8], F32)
    nc.vector.tensor_tensor(P1Q[:, :], S1P, S1P, ALU.mult)
    nc.vector.tensor_tensor(P2Q[:, :], S2P, S2P, ALU.mult)
    nc.vector.tensor_tensor(SSQ[:, :], P1Q[:, :], P2Q[:, :], ALU.add)
    nc.vector.tensor_scalar_mul(E1B[:, :], P1Q[:, :], -ga)
    nc.vector.tensor_scalar_mul(E2B[:, :], P2Q[:, :], -ga)
    nc.vector.tensor_scalar_mul(E1S[:, :], S1P, 2.0 * ga)
    nc.vector.tensor_scalar_mul(E2S[:, :], S2P, 2.0 * ga)
    # AI = -be*TP - ga*SSQ
    nc.vector.tensor_scalar_mul(AI[:, :], SSQ[:, :], -ga)
    nc.vector.scalar_tensor_tensor(AI[:, :], TP, -be, AI[:, :], ALU.mult, ALU.add)
    nc.vector.tensor_scalar(VP[:, :], TP, 0.0, None, ALU.is_gt)
    nc.scalar.activation(EBT[:, :], TP, ACT.Exp, scale=be)
    nc.vector.tensor_tensor(EV[:, :], EBT[:, :], VP[:, :], ALU.mult)
    FJ = ps.tile([128, 1], F32)
    nc.scalar.activation(FJ[:, :], EV[:, 7:8], ACT.Identity)

    RH = [XP3[:, 1024 + b * S:1024 + (b + 1) * S] for b in range(BL)]

    KSUMA = ps.tile([128, 8], F32)
    KSUMB = ps.tile([128, BL], F32)

    # ---- part-1: W = exp(2g*C + a_i), block-causal masked sums -------------
    # c=0: only j<128 can be causal (triangular block); c=1: j<128 all-causal
    # (exp-accum, no mask), j in [128,256) triangular.
    for b in range(BL):
        for c in range(2):
            col = b * 2 + c
            pp1 = pps1.tile([128, S], F32)
            nc.tensor.matmul(
                pp1[:, :],
                XP3[:, col * 128:(col + 1) * 128],
                RH[b],
                start=True, stop=True)
            if c == 0:
                wt = pw.tile([128, 128], F32)
                nc.scalar.activation(
                    wt[:, :], pp1[:, 0:128], ACT.Exp,
                    scale=2.0 * ga, bias=AI[:, col:col + 1])
                wj = pw.tile([128, 128], F32, tag="wjunk")
                nc.vector.scalar_tensor_tensor(
                    wj[:, :], wt[:, :], 1.0,
                    CST[:, C_CAUS + 0:C_CAUS + 128],
                    ALU.mult, ALU.mult,
                    accum_out=KSUMA[:, col:col + 1])
            else:
                wl = pw.tile([128, 128], F32, tag="wleft")
                nc.scalar.activation(
                    wl[:, :], pp1[:, 0:128], ACT.Exp,
                    scale=2.0 * ga, bias=AI[:, col:col + 1],
                    accum_out=KSUMA[:, col:col + 1])
                wt = pw.tile([128, 128], F32)
                nc.scalar.activation(
                    wt[:, :], pp1[:, 128:256], ACT.Exp,
                    scale=2.0 * ga, bias=AI[:, col:col + 1])
                wj = pw.tile([128, 128], F32, tag="wjunk")
                nc.vector.scalar_tensor_tensor(
                    wj[:, :], wt[:, :], 1.0,
                    CST[:, C_CAUS + S + 128:C_CAUS + 2 * S],
                    ALU.mult, ALU.mult,
                    accum_out=KSUMB[:, b:b + 1])

    # ---- part-2 E12 / TK builds (with free row-sum accumulators) ----------
    E1SM = ps.tile([128, 8], F32)
    E2SM = ps.tile([128, 8], F32)
    TKS = ps.tile([128, 8], F32)
    E12 = {}
    TKT = {}
    for b in range(BL):
        for c in range(2):
            col = b * 2 + c
            pm = ppsm.tile([128, 2 * R], F32)
            nc.tensor.matmul(
                pm[:, :], XP3[:, col * 128:(col + 1) * 128],
                XP3[:, 2048:2176], start=True, stop=True)
            e = pe12.tile([128, 2 * R], F32)
            nc.scalar.activation(
                e[:, 0:R], pm[:, 0:R], ACT.Exp,
                bias=E1B[:, col:col + 1], accum_out=E1SM[:, col:col + 1])
            nc.scalar.activation(
                e[:, R:2 * R], pm[:, R:2 * R], ACT.Exp,
                bias=E2B[:, col:col + 1], accum_out=E2SM[:, col:col + 1])
            E12[(b, c)] = e

            tk1 = ptk1.tile([128, R], F32)
            nc.vector.tensor_scalar(
                tk1[:, :], ttg, TP[:, col:col + 1], EV[:, col:col + 1],
                ALU.is_ge, ALU.mult)
            tkt = ps.tile([128, R], BF16, tag=f"tkt{col}")
            nc.vector.scalar_tensor_tensor(
                tkt[:, :], tk1[:, :], 1.0, atk, ALU.mult, ALU.mult,
                accum_out=TKS[:, col:col + 1])
            TKT[(b, c)] = tkt

    # ---- Hadamard Ksp[s, (g1,g2)] = E1[s,g1]*E2[s,g2], quarter-split ------
    POOL_Q = {(k * 32) // 13 for k in range(13)} if POOL_SHARE else set()
    KSP = {}
    for b in range(BL):
        for c in range(2):
            e = E12[(b, c)]
            kt = pksp.tile([128, G], BF16)
            for q in range(4):
                sl = kt[:, q * 1024:(q + 1) * 1024]
                out3 = AP(sl.tensor, sl.offset, [sl.ap[0], [R, 16], [1, R]])
                in_e1 = _rep_inner(e[:, q * 16:(q + 1) * 16], R)
                in_e2 = _rep_outer(e[:, R:2 * R], 16)
                i = (b * 2 + c) * 4 + q
                eng = nc.gpsimd if (i % 32) in POOL_Q else nc.vector
                eng.tensor_tensor(out3, in_e1, in_e2, ALU.mult)
            KSP[(b, c)] = kt

    # ---- part-2 matmuls + fused softplus-accum ----------------------------
    SPS = ps.tile([128, 2], F32)
    for pair in range(2):
        eo = pspo.tile([128, G], F32, tag="eo")
        for q in range(4):
            pp = pps2.tile([128, 1024], F32)
            # pair (i=0, i=1) always on different banks AND different PE
            # column groups -> concurrent; psum groups sequential per bank
            for phase in range(2):
                for c in range(2):
                    for i in range(2):
                        b = pair * 2 + i
                        ns = i ^ phase
                        off = q * 1024 + ns * 512
                        nc.tensor.matmul(
                            pp[i * 64:(i + 1) * 64, ns * 512:(ns + 1) * 512],
                            TKT[(b, c)][:, :],
                            KSP[(b, c)][:, off:off + 512],
                            start=(c == 0), stop=(c == 1),
                            tile_position=(0, i * 64))
            # softplus(z+mu) = (z+mu) + ln(1+exp(-(z+mu))); linear part
            # summed analytically via TKS/E1SM/E2SM below
            nc.scalar.activation(eo[:, q * 1024:(q + 1) * 1024], pp[:, :],
                                 ACT.Exp, scale=-1.0, bias=NMUB[:, :])
        lo = pspo.tile([128, G], F32, tag="lnout")
        nc.scalar.activation(
            lo[:, :], eo[:, :], ACT.Ln, bias=1.0,
            accum_out=SPS[:, pair:pair + 1])

    # ---- part-1 finish: lams = x + ln(1+exp(-x)) + 1e-5, x = al*Ksum+mu ---
    X1 = ps.tile([128, 8], F32)
    nc.vector.tensor_scalar(X1[:, :], KSUMA[:, :], al, mu, ALU.mult, ALU.add)
    x1o = X1[:, 1:2]
    x1odd = AP(x1o.tensor, x1o.offset, [x1o.ap[0], [2, BL]])
    nc.vector.scalar_tensor_tensor(
        x1odd, KSUMB[:, :], al, x1odd, ALU.mult, ALU.add)
    EN1 = ps.tile([128, 8], F32)
    nc.scalar.activation(EN1[:, :], X1[:, :], ACT.Exp, scale=-1.0)
    LN1 = ps.tile([128, 8], F32)
    nc.scalar.activation(LN1[:, :], EN1[:, :], ACT.Ln, bias=1.0)
    LAM = ps.tile([128, 8], F32)
    nc.vector.scalar_tensor_tensor(
        LAM[:, :], X1[:, :], 1e-5, LN1[:, :], ALU.add, ALU.add)
    nc.sync.dma_start(lams8[:, :], LAM[:, :])
    LNL = ps.tile([128, 8], F32)
    nc.scalar.activation(LNL[:, :], LAM[:, :], ACT.Ln)
    nc.vector.tensor_tensor(LNL[:, :], LNL[:, :], VP[:, :], ALU.mult)

    # zsum per (b,c): sum_s TKsum * E1sum * E2sum
    ZS = ps.tile([128, 8], F32)
    nc.vector.tensor_tensor(ZS[:, :], E1SM[:, :], E2SM[:, :], ALU.mult)
    nc.vector.tensor_tensor(ZS[:, :], ZS[:, :], TKS[:, :], ALU.mult)

    SLGS = ppsl.tile([1, 20], F32)
    nc.tensor.matmul(SLGS[0:1, 0:8], ONES[:, :], LNL[:, :], start=True, stop=True)
    nc.tensor.matmul(SLGS[0:1, 12:20], ONES[:, :], ZS[:, :], start=True, stop=True)

    # ---- grid sums: partition-half sums -----------------------------------
    for pair in range(2):
        nc.tensor.matmul(
            SLGS[0:1, 8 + 2 * pair:10 + 2 * pair],
            SPS[:, pair:pair + 1], sel2, start=True, stop=True)

    # ---- loglik = pairsum(SL) - GS*unit_vol -------------------------------
    SLSB = ps.tile([1, 20], F32)
    nc.vector.tensor_copy(SLSB[:, :], SLGS[0:1, :])

    def _pairadd(out_ap, base_off):
        a = SLSB[0:1, base_off:base_off + 8]
        ev = AP(a.tensor, a.offset, [a.ap[0], [2, 4]])
        od = AP(a.tensor, a.offset + 1, [a.ap[0], [2, 4]])
        nc.vector.tensor_tensor(out_ap, ev, od, ALU.add)

    SL4 = ps.tile([1, BL], F32)
    _pairadd(SL4[:, :], 0)          # sum_i ln(lam_i)*mask  per b
    ZS4 = ps.tile([1, BL], F32)
    _pairadd(ZS4[:, :], 12)         # sum z  per b
    # integral_b = ZS4 + mu*T*G + LNACC(b) ; ll = SL4 - UNIT_VOL*integral
    T1 = ps.tile([1, BL], F32)
    nc.vector.tensor_tensor(T1[:, :], ZS4[:, :], SLSB[0:1, 8:12], ALU.add)
    LL = ps.tile([1, BL], F32)
    nc.vector.scalar_tensor_tensor(
        LL[:, :], T1[:, :], -UNIT_VOL, SL4[:, :], ALU.mult, ALU.add)
    nc.vector.tensor_scalar_add(
        LL[:, :], LL[:, :], -UNIT_VOL * mu * float(R) * float(G))
    nc.sync.dma_start(ll4[:, :], LL[:, :])


# ---------------------------------------------------------------------------
_CACHE = {}


def make_inmaps(X: np.ndarray, mu, al, be, ga):
    ss = np.linspace(0.0, 1.0, R, dtype=np.float32)
    tg = np.linspace(0.0, 1.0, R, dtype=np.float32)
    cst = np.zeros((128, NCST), np.float32)
    cst[:, C_SSG:C_SSG + R] = ss
    eg2 = np.exp(-ga * ss * ss).astype(np.float32)
    cst[:, C_EG2:C_EG2 + 2 * R] = np.concatenate([eg2, eg2])
    cst[:, C_TTG:C_TTG + R] = tg
    cst[:, C_ATK:C_ATK + R] = (al * np.exp(-be * tg)).astype(np.float32)
    p = np.arange(128)
    for c in range(2):
        i_idx = c * 128 + p
        cst[:, C_CAUS + c * S:C_CAUS + (c + 1) * S] = (
            np.arange(S)[None, :] < i_idx[:, None]).astype(np.float32)
    cst[:, C_SEL] = (p < 64).astype(np.float32)
    cst[:, C_SEL + 1] = (p >= 64).astype(np.float32)

    in_maps = []
    for k in range(NCORES):
        xk = X[k * BL:(k + 1) * BL]                  # [4, 256, 3]
        xdat = np.ascontiguousarray(xk.transpose(0, 2, 1))  # [4, 3, 256]
        t, s1, s2 = xdat[:, 0], xdat[:, 1], xdat[:, 2]
        # inall = [cst | prt]; prt [128, 24] partition layouts, col = b*2+c
        inall = np.zeros((128, NCST + 24), np.float32)
        inall[:, :NCST] = cst
        for r, arr in ((0, t), (1, s1), (2, s2)):
            inall[:, NCST + r * 8:NCST + (r + 1) * 8] = (
                arr.reshape(BL, 2, 128).transpose(2, 0, 1).reshape(128, 8))
        # xp3 [3, 2048]: cols 0:1024 = [s1g; s2g; ones], 1024: = [s1;s2;bj'] per b
        bj = (be / (2.0 * ga)) * t - 0.5 * (s1 * s1 + s2 * s2) \
            - (1e9 / (2.0 * ga)) * (t <= 0)
        xp3 = np.ones((3, 2176), np.float32)
        xp3[0, :1024] = s1.reshape(-1)
        xp3[1, :1024] = s2.reshape(-1)
        xp3[0, 1024:2048] = s1.reshape(-1)
        xp3[1, 1024:2048] = s2.reshape(-1)
        xp3[2, 1024:2048] = bj.reshape(-1)
        xp3[0, 2048:2112] = 2.0 * ga * ss
        xp3[0, 2112:2176] = 0.0
        xp3[1, 2048:2112] = 0.0
        xp3[1, 2112:2176] = 2.0 * ga * ss
        xp3[2, 2048:2176] = np.tile(-ga * ss * ss, 2)
        in_maps.append({"inall": inall, "xp3": xp3})
    return in_maps


def kernel(X, mu, alpha, beta, gamma):
    X = np.asarray(X, dtype=np.float32)
    mu = float(mu); al = float(alpha); be = float(beta); ga = float(gamma)
    key = (mu, al, be, ga)
    if key not in _CACHE:
        _CACHE[key] = build_program(mu, al, be, ga)
    nc = _CACHE[key]
    in_maps = make_inmaps(X, mu, al, be, ga)
    res = run_bass_kernel_spmd(nc, in_maps, list(range(NCORES))).results
    lams = np.zeros((B, S), np.float32)
    ll = np.zeros((B,), np.float32)
    for k in range(NCORES):
        l8 = res[k]["lams8"]                         # [128, 8]
        lams[k * BL:(k + 1) * BL] = (
            l8.reshape(128, BL, 2).transpose(1, 2, 0).reshape(BL, S))
        ll[k * BL:(k + 1) * BL] = res[k]["ll4"][0]
    return lams, ll
